# revision 28
# baseline (speedup 1.0000x reference)
"""GCNII node regressor on 8 trn2 NeuronCores (Bass/Tile kernel).

Strategy (per sharding_hint): nodes are row-sharded across the 8 cores
(12500 each); edges are partitioned by dst core so the segment-sum is
local; the small weights are replicated.  h lives in [feat, node]
layout.  Per layer each core all-gathers h in 4 source-quarter
sub-AllGathers (pipelined with compute); source windows (half-quarters,
12800 nodes) are DMA'd into SBUF and edge-source columns are gathered
in-SBUF with the stock ap_gather GPSIMD instruction (the runtime image
lacks the custom dma_gather ucode; ap_gather measures ~26ns/row, which
is the kernel's bottleneck).  Gathered message columns are transposed
128-at-a-time on the TensorEngine and aggregated with host-prebuilt
one-hot*weight "S" blocks as dense [K<=128 x 128 dst] matmuls into
PSUM: s = 0.9*Ahat@h + 0.1*h0 accumulates into a resident SBUF tile
[128 feat x nodes]; the layer update h+ = relu((1-b)s + b*s@W) is fused
into one matmul with W_eff = (1-b)I + b*W precomputed on host.

Everything irregular (degrees, edge normalization, sorting edges into
uniform per-(window, dst-block) cells, gather index / S-block streams)
is precomputed on the host in numpy; the device program is
straight-line fp32 and identical on all 8 cores (SPMD).
"""

import math
import os

import numpy as np

# ---------------- problem constants (full size, hardcoded) ----------------
N = 100000
E = 1600000
IN_DIM = 256
HID = 128
LAYERS = 8
ALPHA = 0.1
THETA = 0.5
NCORES = 8

P = 128          # partitions
NW = 8           # source windows per layer (half-quarters)
NI = 2048        # gather slots per ap_gather call
YB = 512         # output block width

LAST_EXEC_NS = None


class Cfg:
    def __init__(self, n, e, in_dim, hid, layers):
        assert n % NCORES == 0
        self.n, self.e, self.in_dim, self.hid, self.layers = n, e, in_dim, hid, layers
        self.n_per = n // NCORES
        self.n_pad = ((self.n_per + P - 1) // P) * P
        # source quarters: q0..q2 of size qs (multiple of 128), q3 remainder
        qs = ((self.n_per + 3) // 4 + P - 1) // P * P
        self.qs = qs
        q3 = self.n_per - 3 * qs
        assert 0 < q3 <= qs, (self.n_per, qs, q3)
        self.qsizes = [qs, qs, qs, q3]          # real rows per rank-quarter
        for sz in self.qsizes:
            assert 4 * sz <= 32767, "window slot must fit int16"
            assert 4 * sz * 1 <= 2 ** 15, "ap_gather num_elems limit"
        self.NB = self.n_pad // P               # dst 128-blocks per core
        assert 3 * qs % P == 0
        self.betas = [float(np.log(THETA / (i + 1) + 1.0)) for i in range(layers)]

    def wsize(self, w):
        """window w (= quarter q, rank half h) node count"""
        return 4 * self.qsizes[w // 2]


def _cfg_full():
    return Cfg(N, E, IN_DIM, HID, LAYERS)


# ---------------- host preprocessing ----------------

def preprocess(x, edge_index, W_in, b_in, convs_W, W_out, b_out, cfg):
    """Build per-core input maps + shared structure metadata."""
    n, n_per, qs = cfg.n, cfg.n_per, cfg.qs
    qsz = np.asarray(cfg.qsizes, np.int64)
    row = np.asarray(edge_index[0], np.int64)
    col = np.asarray(edge_index[1], np.int64)

    deg = np.bincount(col, minlength=n).astype(np.float32) + 1.0
    dinv = (1.0 / np.sqrt(deg)).astype(np.float32)
    wt = ((1.0 - ALPHA) * dinv[row] * dinv[col]).astype(np.float32)

    # append self loops as explicit edges
    allv = np.arange(n, dtype=np.int64)
    row_a = np.concatenate([row, allv])
    col_a = np.concatenate([col, allv])
    wt_a = np.concatenate([wt, ((1.0 - ALPHA) * dinv * dinv).astype(np.float32)])

    # source window + slot within window
    r_s = row_a // n_per
    i_s = row_a % n_per
    q_s = np.minimum(i_s // qs, 3)
    w_s = 2 * q_s + (r_s >= 4)
    slot = ((r_s % 4) * qsz[q_s] + (i_s - q_s * qs)).astype(np.int64)

    r_d = col_a // n_per
    dloc = col_a % n_per
    b_idx = dloc // P                            # dst 128-block
    dcol = (dloc % P).astype(np.int64)

    NB = cfg.NB
    counts = np.zeros((NCORES, NW, NB), np.int64)
    np.add.at(counts, (r_d, w_s, b_idx), 1)
    n_cb = counts.max(axis=0)                    # [NW, NB]
    n_cb = np.maximum(32 * ((n_cb + 31) // 32), 32)
    L = n_cb.sum(axis=1)
    n_cb[:, NB - 1] += (-L) % P                  # chunk streams multiple of 128
    L = n_cb.sum(axis=1)                         # [NW]

    offs = np.zeros((NW, NB + 1), np.int64)
    offs[:, 1:] = np.cumsum(n_cb, axis=1)

    key = (r_d * NW + w_s) * NB + b_idx
    order = np.argsort(key, kind="stable")
    sk = key[order]
    grp_first = np.r_[0, np.flatnonzero(np.diff(sk)) + 1]
    grp_id = np.zeros(len(sk), np.int64)
    grp_id[grp_first[1:]] = 1
    grp_id = np.cumsum(grp_id)
    rank_in_cell = np.arange(len(sk)) - grp_first[grp_id]
    pos = offs[w_s[order], b_idx[order]] + rank_in_cell

    in_maps = []
    for r in range(NCORES):
        m = {}
        xs = np.zeros((cfg.n_pad, cfg.in_dim), np.float32)
        xs[:n_per] = np.asarray(x[r * n_per:(r + 1) * n_per], np.float32)
        m["x"] = xs
        sel_r = r_d[order] == r
        for c in range(NW):
            sel = sel_r & (w_s[order] == c)
            p = pos[sel]
            idx_stream = np.zeros(L[c], np.int16)
            idx_stream[p] = slot[order][sel].astype(np.int16)
            sarr = np.zeros((P, (L[c] // P) * P), np.float32)
            sarr[p % P, (p // P) * P + dcol[order][sel]] = wt_a[order][sel]
            idxd = np.tile(idx_stream.reshape(-1, 16).T, (8, 1))
            m[f"idx{c}"] = np.ascontiguousarray(idxd)
            m[f"sblk{c}"] = sarr
        m["w_in"] = np.asarray(W_in, np.float32)
        m["b_in"] = np.asarray(b_in, np.float32).reshape(cfg.hid, 1)
        weff = np.concatenate(
            [((1.0 - cfg.betas[i]) * np.eye(cfg.hid, dtype=np.float32)
              + cfg.betas[i] * np.asarray(convs_W[i], np.float32))
             for i in range(cfg.layers)], axis=1)
        m["w_eff"] = weff
        m["w_out"] = np.asarray(W_out, np.float32).reshape(cfg.hid, 1)
        m["b_out"] = np.asarray(b_out, np.float32).reshape(1, 1)
        in_maps.append(m)

    return in_maps, {"n_cb": n_cb, "L": L}


# ---------------- device program ----------------

def build(cfg, meta, debug=False):
    import concourse.bass as bass
    import concourse.mybir as mybir
    from concourse import bacc
    from concourse.masks import make_identity
    from concourse.tile import TileContext
    from contextlib import ExitStack

    f32 = mybir.dt.float32
    i16 = mybir.dt.int16
    Relu = mybir.ActivationFunctionType.Relu
    n_cb, L = meta["n_cb"], meta["L"]
    hid, in_dim = cfg.hid, cfg.in_dim
    NB = cfg.NB
    qsz = cfg.qsizes
    SB = 16                  # S groups per stream tile

    nc = bacc.Bacc("TRN2", target_bir_lowering=False, debug=debug)

    x_in = nc.dram_tensor("x", [cfg.n_pad, in_dim], f32, kind="ExternalInput")
    idx_in, s_in = [], []
    for c in range(NW):
        idx_in.append(nc.dram_tensor(f"idx{c}", [P, int(L[c]) // 16], i16,
                                     kind="ExternalInput"))
        s_in.append(nc.dram_tensor(f"sblk{c}", [P, (int(L[c]) // P) * P], f32,
                                   kind="ExternalInput"))
    w_in_t = nc.dram_tensor("w_in", [in_dim, hid], f32, kind="ExternalInput")
    b_in_t = nc.dram_tensor("b_in", [hid, 1], f32, kind="ExternalInput")
    w_eff_t = nc.dram_tensor("w_eff", [hid, cfg.layers * hid], f32,
                             kind="ExternalInput")
    w_out_t = nc.dram_tensor("w_out", [hid, 1], f32, kind="ExternalInput")
    b_out_t = nc.dram_tensor("b_out", [1, 1], f32, kind="ExternalInput")
    y_out = nc.dram_tensor("y", [1, cfg.n_pad], f32, kind="ExternalOutput")

    rg = [list(range(NCORES))]

    with TileContext(nc) as tc, ExitStack() as ctx:
        const = ctx.enter_context(tc.tile_pool(name="const", bufs=1))
        resid = ctx.enter_context(tc.tile_pool(name="resid", bufs=1))
        winp = ctx.enter_context(tc.tile_pool(name="winp", bufs=1))
        gathp = ctx.enter_context(tc.tile_pool(name="gath", bufs=3))
        mtp = ctx.enter_context(tc.tile_pool(name="mtp", bufs=20))
        sblkp = ctx.enter_context(tc.tile_pool(name="sblk", bufs=3))
        idxp = ctx.enter_context(tc.tile_pool(name="idxt", bufs=3))
        xiop = ctx.enter_context(tc.tile_pool(name="xio", bufs=2))
        xtp = ctx.enter_context(tc.tile_pool(name="xt", bufs=3))
        wbp = ctx.enter_context(tc.tile_pool(name="wb", bufs=3))
        h0sp = ctx.enter_context(tc.tile_pool(name="h0sp", bufs=3))
        ytp = ctx.enter_context(tc.tile_pool(name="yt", bufs=2))
        pagg = ctx.enter_context(tc.tile_pool(name="pagg", bufs=3, space="PSUM"))
        ptr = ctx.enter_context(tc.tile_pool(name="ptr", bufs=3, space="PSUM"))
        pmisc = ctx.enter_context(tc.tile_pool(name="pmisc", bufs=2, space="PSUM"))
        dram = ctx.enter_context(tc.tile_pool(name="dram", bufs=1, space="DRAM"))

        # per-quarter h shard in [feat, node] layout + AG'd tables
        h_shard = []
        h_table = [None] * 4
        for q in range(4):
            h_shard.append(dram.tile([P, qsz[q]], f32, tag=f"h_shard{q}",
                                     name=f"h_shard{q}"))
        h0s_dram = dram.tile([P, cfg.n_pad], f32, tag="h0s", name="h0s_dram")

        id1 = const.tile([P, P], f32, tag="id1", name="id1")
        make_identity(nc, id1[:])
        w_in_sb = const.tile([P, (in_dim // P) * hid], f32, tag="w_in",
                             name="w_in_sb")
        for k in range(in_dim // P):
            nc.sync.dma_start(out=w_in_sb[:, k * hid:(k + 1) * hid],
                              in_=w_in_t[k * P:(k + 1) * P, :])
        b_in_sb = const.tile([P, 1], f32, tag="b_in", name="b_in_sb")
        nc.sync.dma_start(out=b_in_sb[:], in_=b_in_t[:])
        b_in_s = const.tile([P, 1], f32, tag="b_in_s", name="b_in_s")
        nc.vector.tensor_scalar_mul(b_in_s[:], b_in_sb[:], ALPHA)
        w_eff_sb = const.tile([P, cfg.layers * hid], f32, tag="w_eff",
                              name="w_eff_sb")
        nc.sync.dma_start(out=w_eff_sb[:], in_=w_eff_t[:])
        w_out_sb = const.tile([P, 1], f32, tag="w_out", name="w_out_sb")
        nc.sync.dma_start(out=w_out_sb[:], in_=w_out_t[:])
        b_out_sb = const.tile([1, 1], f32, tag="b_out", name="b_out_sb")
        nc.sync.dma_start(out=b_out_sb[:], in_=b_out_t[:])

        s_acc = resid.tile([P, cfg.n_pad], f32, tag="s_acc", name="s_acc")

        reg_cache = {}

        def nreg(v):
            if v not in reg_cache:
                reg_cache[v] = nc.gpsimd.to_reg(v)
            return reg_cache[v]

        def emit_ag(q):
            tab = dram.tile([NCORES * P, qsz[q]], f32, tag=f"h_table{q}",
                            name=f"h_table{q}", addr_space="Shared", bufs=2)
            nc.gpsimd.collective_compute(
                "AllGather", mybir.AluOpType.bypass, replica_groups=rg,
                ins=[h_shard[q][:, 0:qsz[q]].opt()],
                outs=[tab[:].opt()])
            h_table[q] = tab

        def shard_cols(blk, width):
            """node block (width cols from blk*width) -> (quarter, col off)"""
            lo = blk * width
            q = min(lo // cfg.qs, 3)
            return q, lo - q * cfg.qs

        # ---------------- init: h0 = relu(x@W_in + b_in) ----------------
        for nt in range(NB):
            x_tile = xiop.tile([P, in_dim], f32, tag="x", name="x_tile")
            nc.sync.dma_start(out=x_tile[:], in_=x_in[nt * P:(nt + 1) * P, :])
            xts = []
            for k in range(in_dim // P):
                xt_ps = ptr.tile([P, P], f32, tag="ptr", name="xt_ps")
                nc.tensor.transpose(xt_ps[:], x_tile[:, k * P:(k + 1) * P], id1[:])
                xt_sb = xtp.tile([P, P], f32, tag="xt", name="xt_sb")
                nc.vector.tensor_copy(out=xt_sb[:], in_=xt_ps[:])
                xts.append(xt_sb)
            ph0 = pmisc.tile([P, YB], f32, tag="pmisc", name="pm")
            nk = in_dim // P
            for k in range(nk):
                nc.tensor.matmul(ph0[:, :P], lhsT=w_in_sb[:, k * hid:(k + 1) * hid],
                                 rhs=xts[k][:], start=(k == 0), stop=(k == nk - 1))
            wb = wbp.tile([P, YB], f32, tag="wb", name="wb")
            nc.scalar.activation(wb[:, :P], ph0[:, :P], Relu, bias=b_in_sb[:])
            q, co = shard_cols(nt, P)
            take = min(P, qsz[q] - co)
            nc.sync.dma_start(out=h_shard[q][:, co:co + take], in_=wb[:, :take])
            h0t = h0sp.tile([P, P], f32, tag="h0t", name="h0t")
            nc.scalar.activation(h0t[:], ph0[:, :P], Relu, bias=b_in_s[:],
                                 scale=ALPHA)
            nc.sync.dma_start(out=h0s_dram[:, nt * P:(nt + 1) * P], in_=h0t[:])
            if q < 3 and nt == (q + 1) * (cfg.qs // P) - 1:
                emit_ag(q)
            elif nt == NB - 1:
                emit_ag(3)

        # ---------------- layers ----------------
        for layer in range(cfg.layers):
            last = layer == cfg.layers - 1
            tables = list(h_table)

            def finish_block(b):
                """512-wide output block b of s_acc is complete"""
                w = min(YB, cfg.n_pad - b * YB)
                cols = slice(b * YB, b * YB + w)
                ps = pmisc.tile([P, YB], f32, tag="pmisc", name="pm")
                nc.tensor.matmul(ps[:, :w],
                                 lhsT=w_eff_sb[:, layer * hid:(layer + 1) * hid],
                                 rhs=s_acc[:, cols], start=True, stop=True)
                if not last:
                    wb = wbp.tile([P, YB], f32, tag="wb", name="wb")
                    nc.scalar.activation(wb[:, :w], ps[:, :w], Relu)
                    done = 0
                    while done < w:       # may straddle quarter boundary
                        q = min((b * YB + done) // cfg.qs, 3)
                        co = b * YB + done - q * cfg.qs
                        take = min(w - done, qsz[q] - co)
                        if take <= 0:     # pad columns past the real nodes
                            break
                        nc.sync.dma_start(out=h_shard[q][:, co:co + take],
                                          in_=wb[:, done:done + take])
                        done += take
                    # fire AGs when a quarter's columns are all written
                    hi = b * YB + w
                    for q in range(3):
                        if b * YB < (q + 1) * cfg.qs <= hi:
                            emit_ag(q)
                    if hi == cfg.n_pad:
                        emit_ag(3)
                else:
                    h8 = wbp.tile([P, YB], f32, tag="wb", name="wb")
                    nc.scalar.activation(h8[:, :w], ps[:, :w], Relu)
                    psy = pmisc.tile([P, YB], f32, tag="pmisc", name="pm")
                    nc.tensor.matmul(psy[0:1, :w], lhsT=w_out_sb[:, 0:1],
                                     rhs=h8[:, :w], start=True, stop=True)
                    yt = ytp.tile([1, YB], f32, tag="yt", name="yt")
                    nc.vector.tensor_tensor(
                        out=yt[0:1, :w], in0=psy[0:1, :w],
                        in1=b_out_sb[0:1, 0:1].to_broadcast([1, w]),
                        op=mybir.AluOpType.add)
                    nc.sync.dma_start(out=y_out[0:1, b * YB:b * YB + w],
                                      in_=yt[0:1, :w])

            for c in range(NW):
                q = c // 2
                wsz = cfg.wsize(c)
                # load window: 4 rank blocks of the AG'd quarter table
                win = winp.tile([P, 4 * cfg.qs], f32, tag="win", name="win")
                for j in range(4):
                    rr = (c % 2) * 4 + j
                    nc.sync.dma_start(
                        out=win[:, j * qsz[q]:(j + 1) * qsz[q]],
                        in_=tables[q][rr * P:(rr + 1) * P, :])
                win3 = win[:, :wsz].rearrange("p (n d) -> p n d", d=1)

                Lc = int(L[c])
                nsg = (Lc + NI - 1) // NI
                mt_tiles = {}          # 128-slot group -> sbuf [slot, feat]
                s_tiles = [None] * ((Lc // P + SB - 1) // SB)

                def ensure_group(g, c=c, win3=win3, Lc=Lc, mt_tiles=mt_tiles,
                                 s_tiles=s_tiles):
                    """gather+transpose 128-slot group g; returns sbuf tiles"""
                    if g in mt_tiles:
                        return
                    sg = (g * P) // NI
                    slots = min(NI, Lc - sg * NI)
                    gt = gathp.tile([P, NI], f32, tag="gt", name="gt")
                    it = idxp.tile([P, NI // 16], i16, tag="it", name="it")
                    nc.sync.dma_start(
                        out=it[:, :slots // 16],
                        in_=idx_in[c][:, sg * (NI // 16):sg * (NI // 16) + slots // 16])
                    nc.gpsimd.ap_gather(
                        gt[:, :slots].rearrange("p (n d) -> p n d", d=1), win3,
                        it[:, :slots // 16], P, wsz, 1, slots)
                    sbi = (sg * NI) // (SB * P)
                    for gg in range(sg * (NI // P), sg * (NI // P) + slots // P):
                        ps = ptr.tile([P, P], f32, tag="ptr", name="tr_ps")
                        off = gg * P - sg * NI
                        nc.tensor.transpose(ps[:], gt[:, off:off + P], id1[:])
                        mt = mtp.tile([P, P], f32, tag="mt", name="mt")
                        nc.vector.tensor_copy(out=mt[:], in_=ps[:])
                        mt_tiles[gg] = mt
                    for sb in range(sbi, min(sbi + NI // (SB * P) + 1,
                                             len(s_tiles))):
                        if s_tiles[sb] is None:
                            st = sblkp.tile([P, SB * P], f32, tag="st", name="st")
                            lo = sb * SB * P
                            ncols = min(SB * P, (Lc // P) * P - lo)
                            nc.sync.dma_start(out=st[:, :ncols],
                                              in_=s_in[c][:, lo:lo + ncols])
                            s_tiles[sb] = st

                cur = 0
                for b in range(NB):
                    ps_b = pagg.tile([P, P], f32, tag="ps_b", name="ps_b")
                    n_slots = int(n_cb[c][b])
                    first = True
                    left = n_slots
                    while left > 0:
                        g, p0 = cur // P, cur % P
                        ln = 0
                        for sz in (128, 64, 32):
                            if p0 % sz == 0 and left >= sz and p0 + sz <= P:
                                ln = sz
                                break
                        assert ln, (p0, left)
                        ensure_group(g)
                        st = s_tiles[g // SB]
                        so = (g % SB) * P
                        nc.tensor.matmul(
                            ps_b[:],
                            lhsT=mt_tiles[g][p0:p0 + ln, :],
                            rhs=st[p0:p0 + ln, so:so + P],
                            start=first, stop=(ln == left),
                            tile_position=(p0, 0))
                        first = False
                        cur += ln
                        left -= ln
                    if c == 0:
                        h0t = h0sp.tile([P, P], f32, tag="h0t", name="h0t")
                        nc.sync.dma_start(out=h0t[:],
                                          in_=h0s_dram[:, b * P:(b + 1) * P])
                        nc.vector.tensor_add(out=s_acc[:, b * P:(b + 1) * P],
                                             in0=h0t[:], in1=ps_b[:])
                    else:
                        nc.vector.tensor_add(out=s_acc[:, b * P:(b + 1) * P],
                                             in0=s_acc[:, b * P:(b + 1) * P],
                                             in1=ps_b[:])
                    if c == NW - 1 and (((b + 1) * P) % YB == 0 or b == NB - 1):
                        finish_block((b * P) // YB)

    nc.compile()
    return nc


# ---------------- top level ----------------

def _assemble_y(results, cfg):
    parts = []
    for r in range(NCORES):
        y = np.asarray(results[r]["y"], np.float32).reshape(-1)
        parts.append(y[:cfg.n_per])
    return np.concatenate(parts)


def _run_pjrt(nc, in_maps, n_cores, time_iters=0):
    """Execute the bass program on the NeuronCores via PJRT (the axon
    redirect path of run_bass_kernel_spmd), with inputs pre-staged on
    device.  Mirrors concourse.bass2jax.run_bass_via_pjrt (multi-core).

    The axon dispatch floor is ~80ms/call, so single-call wall time says
    nothing about device time; with time_iters > 0 the marginal cost per
    execute between pipelined batches of M_lo and M_hi back-to-back
    calls is reported: device exec time plus ~1ms per-call dispatch (an
    honest upper bound on HW time).
    """
    import time
    import jax
    from jax.sharding import Mesh, NamedSharding, PartitionSpec
    from jax.experimental.shard_map import shard_map
    from concourse import bass2jax, mybir

    bass2jax.install_neuronx_cc_hook()

    partition_name = nc.partition_id_tensor.name if nc.partition_id_tensor else None
    in_names, out_names, out_avals, zero_outs = [], [], [], []
    for alloc in nc.m.functions[0].allocations:
        if not isinstance(alloc, mybir.MemoryLocationSet):
            continue
        name = alloc.memorylocations[0].name
        if alloc.kind == "ExternalInput":
            if name != partition_name:
                in_names.append(name)
        elif alloc.kind == "ExternalOutput":
            out_names.append(name)
            shape = tuple(alloc.tensor_shape)
            dtype = mybir.dt.np(alloc.dtype)
            out_avals.append(jax.core.ShapedArray(shape, dtype))
            zero_outs.append(np.zeros(shape, dtype))
    n_params = len(in_names)
    n_outs = len(out_avals)
    in_names.extend(out_names)
    if partition_name is not None:
        in_names.append(partition_name)
    donate = tuple(range(n_params, n_params + n_outs))

    def _body(*args):
        operands = list(args)
        if partition_name is not None:
            operands.append(bass2jax.partition_id_tensor())
        outs = bass2jax._bass_exec_p.bind(
            *operands,
            out_avals=tuple(out_avals),
            in_names=tuple(in_names),
            out_names=tuple(out_names),
            lowering_input_output_aliases=(),
            sim_require_finite=True,
            sim_require_nnan=True,
            nc=nc,
        )
        return tuple(outs)

    devices = jax.devices()[:n_cores]
    assert len(devices) == n_cores
    mesh = Mesh(np.asarray(devices), ("core",))
    in_specs = (PartitionSpec("core"),) * (n_params + n_outs)
    out_specs = (PartitionSpec("core"),) * len(out_names)
    sharded = jax.jit(
        shard_map(_body, mesh=mesh, in_specs=in_specs, out_specs=out_specs,
                  check_rep=False),
        donate_argnums=donate, keep_unused=True)

    shard = NamedSharding(mesh, PartitionSpec("core"))
    concat_in = [
        jax.device_put(
            np.concatenate([np.asarray(in_maps[c][name]) for c in range(n_cores)],
                           axis=0), shard)
        for name in in_names[:n_params]
    ]
    jax.block_until_ready(concat_in)

    def zeros():
        return [
            jax.device_put(np.zeros((n_cores * z.shape[0], *z.shape[1:]), z.dtype),
                           shard)
            for z in zero_outs
        ]

    out_arrs = jax.block_until_ready(sharded(*concat_in, *zeros()))
    exec_ns = None
    if time_iters > 0:
        m_lo, m_hi = 4, 4 + max(4, time_iters)

        def run_m(m):
            zs = [zeros() for _ in range(m)]
            jax.block_until_ready(zs)
            t0 = time.perf_counter()
            rs = [sharded(*concat_in, *z) for z in zs]
            jax.block_until_ready(rs)
            return time.perf_counter() - t0

        run_m(2)  # warm
        lo = min(run_m(m_lo) for _ in range(2))
        hi = min(run_m(m_hi) for _ in range(2))
        exec_ns = int(max(hi - lo, 0) / (m_hi - m_lo) * 1e9)
    results = [
        {name: np.asarray(out_arrs[i]).reshape(n_cores, *out_avals[i].shape)[c]
         for i, name in enumerate(out_names)}
        for c in range(n_cores)
    ]
    return results, exec_ns


def kernel(x, edge_index, W_in, b_in, convs_W, W_out, b_out):
    global LAST_EXEC_NS
    cfg = _cfg_full()
    in_maps, meta = preprocess(x, edge_index, W_in, b_in, convs_W, W_out, b_out,
                               cfg)
    nc = build(cfg, meta)
    iters = int(os.environ.get("KERNEL_TIME_ITERS", "0"))
    results, exec_ns = _run_pjrt(nc, in_maps, NCORES, time_iters=iters)
    LAST_EXEC_NS = exec_ns
    return _assemble_y(results, cfg)


# revision 30
# speedup vs baseline: 1.0306x; 1.0306x over previous
"""GCNII node regressor on 8 trn2 NeuronCores (Bass/Tile kernel).

Strategy (per sharding_hint): nodes are row-sharded across the 8 cores
(12500 each); edges are partitioned by dst core so the segment-sum is
local; the small weights are replicated.  h lives in [feat, node]
layout.  Per layer each core all-gathers h in 4 source-quarter
sub-AllGathers (pipelined with compute); source windows (half-quarters,
12800 nodes) are DMA'd into SBUF and edge-source columns are gathered
in-SBUF with the stock ap_gather GPSIMD instruction (the runtime image
lacks the custom dma_gather ucode; ap_gather measures ~26ns/row, which
is the kernel's bottleneck).  Gathered message columns are transposed
128-at-a-time on the TensorEngine and aggregated with host-prebuilt
one-hot*weight "S" blocks as dense [K<=128 x 128 dst] matmuls into
PSUM: s = 0.9*Ahat@h + 0.1*h0 accumulates into a resident SBUF tile
[128 feat x nodes]; the layer update h+ = relu((1-b)s + b*s@W) is fused
into one matmul with W_eff = (1-b)I + b*W precomputed on host.

Everything irregular (degrees, edge normalization, sorting edges into
uniform per-(window, dst-block) cells, gather index / S-block streams)
is precomputed on the host in numpy; the device program is
straight-line fp32 and identical on all 8 cores (SPMD).
"""

import math
import os

import numpy as np

# ---------------- problem constants (full size, hardcoded) ----------------
N = 100000
E = 1600000
IN_DIM = 256
HID = 128
LAYERS = 8
ALPHA = 0.1
THETA = 0.5
NCORES = 8

P = 128          # partitions
NW = 8           # source windows per layer (half-quarters)
NI = 2048        # gather slots per ap_gather call
CW = 256         # aggregation cell dst width
YB = 512         # output block width

LAST_EXEC_NS = None


class Cfg:
    def __init__(self, n, e, in_dim, hid, layers):
        assert n % NCORES == 0
        self.n, self.e, self.in_dim, self.hid, self.layers = n, e, in_dim, hid, layers
        self.n_per = n // NCORES
        self.n_pad = ((self.n_per + P - 1) // P) * P
        # source quarters: q0..q2 of size qs (multiple of 128), q3 remainder
        qs = ((self.n_per + 3) // 4 + P - 1) // P * P
        self.qs = qs
        q3 = self.n_per - 3 * qs
        assert 0 < q3 <= qs, (self.n_per, qs, q3)
        self.qsizes = [qs, qs, qs, q3]          # real rows per rank-quarter
        for sz in self.qsizes:
            assert 4 * sz <= 32767, "window slot must fit int16"
            assert 4 * sz * 1 <= 2 ** 15, "ap_gather num_elems limit"
        self.NB = self.n_pad // P               # dst 128-blocks per core
        self.NC2 = self.n_pad // CW             # dst cell blocks per core
        assert 3 * qs % P == 0
        self.betas = [float(np.log(THETA / (i + 1) + 1.0)) for i in range(layers)]

    def wsize(self, w):
        """window w (= quarter q, rank half h) node count"""
        return 4 * self.qsizes[w // 2]


def _cfg_full():
    return Cfg(N, E, IN_DIM, HID, LAYERS)


# ---------------- host preprocessing ----------------

def preprocess(x, edge_index, W_in, b_in, convs_W, W_out, b_out, cfg):
    """Build per-core input maps + shared structure metadata."""
    n, n_per, qs = cfg.n, cfg.n_per, cfg.qs
    qsz = np.asarray(cfg.qsizes, np.int64)
    row = np.asarray(edge_index[0], np.int64)
    col = np.asarray(edge_index[1], np.int64)

    deg = np.bincount(col, minlength=n).astype(np.float32) + 1.0
    dinv = (1.0 / np.sqrt(deg)).astype(np.float32)
    wt = ((1.0 - ALPHA) * dinv[row] * dinv[col]).astype(np.float32)

    # append self loops as explicit edges
    allv = np.arange(n, dtype=np.int64)
    row_a = np.concatenate([row, allv])
    col_a = np.concatenate([col, allv])
    wt_a = np.concatenate([wt, ((1.0 - ALPHA) * dinv * dinv).astype(np.float32)])

    # source window + slot within window
    r_s = row_a // n_per
    i_s = row_a % n_per
    q_s = np.minimum(i_s // qs, 3)
    w_s = 2 * q_s + (r_s >= 4)
    slot = ((r_s % 4) * qsz[q_s] + (i_s - q_s * qs)).astype(np.int64)

    r_d = col_a // n_per
    dloc = col_a % n_per
    b_idx = dloc // CW                           # dst cell block
    dcol = (dloc % CW).astype(np.int64)

    NB = cfg.NC2
    counts = np.zeros((NCORES, NW, NB), np.int64)
    np.add.at(counts, (r_d, w_s, b_idx), 1)
    n_cb = counts.max(axis=0)                    # [NW, NB]
    n_cb = np.maximum(32 * ((n_cb + 31) // 32), 32)
    L = n_cb.sum(axis=1)
    n_cb[:, NB - 1] += (-L) % P                  # chunk streams multiple of 128
    L = n_cb.sum(axis=1)                         # [NW]

    offs = np.zeros((NW, NB + 1), np.int64)
    offs[:, 1:] = np.cumsum(n_cb, axis=1)

    key = (r_d * NW + w_s) * NB + b_idx
    order = np.argsort(key, kind="stable")
    sk = key[order]
    grp_first = np.r_[0, np.flatnonzero(np.diff(sk)) + 1]
    grp_id = np.zeros(len(sk), np.int64)
    grp_id[grp_first[1:]] = 1
    grp_id = np.cumsum(grp_id)
    rank_in_cell = np.arange(len(sk)) - grp_first[grp_id]
    pos = offs[w_s[order], b_idx[order]] + rank_in_cell

    in_maps = []
    for r in range(NCORES):
        m = {}
        xs = np.zeros((cfg.n_pad, cfg.in_dim), np.float32)
        xs[:n_per] = np.asarray(x[r * n_per:(r + 1) * n_per], np.float32)
        m["x"] = xs
        sel_r = r_d[order] == r
        for c in range(NW):
            sel = sel_r & (w_s[order] == c)
            p = pos[sel]
            idx_stream = np.zeros(L[c], np.int16)
            idx_stream[p] = slot[order][sel].astype(np.int16)
            sarr = np.zeros((P, (L[c] // P) * CW), np.float32)
            sarr[p % P, (p // P) * CW + dcol[order][sel]] = wt_a[order][sel]
            idxd = np.tile(idx_stream.reshape(-1, 16).T, (8, 1))
            m[f"idx{c}"] = np.ascontiguousarray(idxd)
            m[f"sblk{c}"] = sarr
        m["w_in"] = np.asarray(W_in, np.float32)
        m["b_in"] = np.asarray(b_in, np.float32).reshape(cfg.hid, 1)
        weff = np.concatenate(
            [((1.0 - cfg.betas[i]) * np.eye(cfg.hid, dtype=np.float32)
              + cfg.betas[i] * np.asarray(convs_W[i], np.float32))
             for i in range(cfg.layers)], axis=1)
        m["w_eff"] = weff
        m["w_out"] = np.asarray(W_out, np.float32).reshape(cfg.hid, 1)
        m["b_out"] = np.asarray(b_out, np.float32).reshape(1, 1)
        in_maps.append(m)

    return in_maps, {"n_cb": n_cb, "L": L}


# ---------------- device program ----------------

def build(cfg, meta, debug=False):
    import concourse.bass as bass
    import concourse.mybir as mybir
    from concourse import bacc
    from concourse.masks import make_identity
    from concourse.tile import TileContext
    from contextlib import ExitStack

    f32 = mybir.dt.float32
    i16 = mybir.dt.int16
    Relu = mybir.ActivationFunctionType.Relu
    n_cb, L = meta["n_cb"], meta["L"]
    hid, in_dim = cfg.hid, cfg.in_dim
    NB = cfg.NB
    qsz = cfg.qsizes
    SB = 8                   # S groups per stream tile

    nc = bacc.Bacc("TRN2", target_bir_lowering=False, debug=debug)

    x_in = nc.dram_tensor("x", [cfg.n_pad, in_dim], f32, kind="ExternalInput")
    idx_in, s_in = [], []
    for c in range(NW):
        idx_in.append(nc.dram_tensor(f"idx{c}", [P, int(L[c]) // 16], i16,
                                     kind="ExternalInput"))
        s_in.append(nc.dram_tensor(f"sblk{c}", [P, (int(L[c]) // P) * CW], f32,
                                   kind="ExternalInput"))
    w_in_t = nc.dram_tensor("w_in", [in_dim, hid], f32, kind="ExternalInput")
    b_in_t = nc.dram_tensor("b_in", [hid, 1], f32, kind="ExternalInput")
    w_eff_t = nc.dram_tensor("w_eff", [hid, cfg.layers * hid], f32,
                             kind="ExternalInput")
    w_out_t = nc.dram_tensor("w_out", [hid, 1], f32, kind="ExternalInput")
    b_out_t = nc.dram_tensor("b_out", [1, 1], f32, kind="ExternalInput")
    y_out = nc.dram_tensor("y", [1, cfg.n_pad], f32, kind="ExternalOutput")

    rg = [list(range(NCORES))]

    with TileContext(nc) as tc, ExitStack() as ctx:
        const = ctx.enter_context(tc.tile_pool(name="const", bufs=1))
        resid = ctx.enter_context(tc.tile_pool(name="resid", bufs=1))
        winp = ctx.enter_context(tc.tile_pool(name="winp", bufs=1))
        gathp = ctx.enter_context(tc.tile_pool(name="gath", bufs=3))
        mtp = ctx.enter_context(tc.tile_pool(name="mtp", bufs=20))
        sblkp = ctx.enter_context(tc.tile_pool(name="sblk", bufs=3))
        idxp = ctx.enter_context(tc.tile_pool(name="idxt", bufs=3))
        xiop = ctx.enter_context(tc.tile_pool(name="xio", bufs=2))
        xtp = ctx.enter_context(tc.tile_pool(name="xt", bufs=3))
        wbp = ctx.enter_context(tc.tile_pool(name="wb", bufs=3))
        h0sp = ctx.enter_context(tc.tile_pool(name="h0sp", bufs=3))
        ytp = ctx.enter_context(tc.tile_pool(name="yt", bufs=2))
        pagg = ctx.enter_context(tc.tile_pool(name="pagg", bufs=3, space="PSUM"))
        ptr = ctx.enter_context(tc.tile_pool(name="ptr", bufs=3, space="PSUM"))
        pmisc = ctx.enter_context(tc.tile_pool(name="pmisc", bufs=2, space="PSUM"))
        dram = ctx.enter_context(tc.tile_pool(name="dram", bufs=1, space="DRAM"))

        # per-quarter h shard in [feat, node] layout + AG'd tables
        h_shard = []
        h_table = [None] * 4
        for q in range(4):
            h_shard.append(dram.tile([P, qsz[q]], f32, tag=f"h_shard{q}",
                                     name=f"h_shard{q}"))
        h0s_dram = dram.tile([P, cfg.n_pad], f32, tag="h0s", name="h0s_dram")

        id1 = const.tile([P, P], f32, tag="id1", name="id1")
        make_identity(nc, id1[:])
        w_in_sb = const.tile([P, (in_dim // P) * hid], f32, tag="w_in",
                             name="w_in_sb")
        for k in range(in_dim // P):
            nc.sync.dma_start(out=w_in_sb[:, k * hid:(k + 1) * hid],
                              in_=w_in_t[k * P:(k + 1) * P, :])
        b_in_sb = const.tile([P, 1], f32, tag="b_in", name="b_in_sb")
        nc.sync.dma_start(out=b_in_sb[:], in_=b_in_t[:])
        b_in_s = const.tile([P, 1], f32, tag="b_in_s", name="b_in_s")
        nc.vector.tensor_scalar_mul(b_in_s[:], b_in_sb[:], ALPHA)
        w_eff_sb = const.tile([P, cfg.layers * hid], f32, tag="w_eff",
                              name="w_eff_sb")
        nc.sync.dma_start(out=w_eff_sb[:], in_=w_eff_t[:])
        w_out_sb = const.tile([P, 1], f32, tag="w_out", name="w_out_sb")
        nc.sync.dma_start(out=w_out_sb[:], in_=w_out_t[:])
        b_out_sb = const.tile([1, 1], f32, tag="b_out", name="b_out_sb")
        nc.sync.dma_start(out=b_out_sb[:], in_=b_out_t[:])

        s_acc = resid.tile([P, cfg.n_pad], f32, tag="s_acc", name="s_acc")

        reg_cache = {}

        def nreg(v):
            if v not in reg_cache:
                reg_cache[v] = nc.gpsimd.to_reg(v)
            return reg_cache[v]

        def emit_ag(q):
            tab = dram.tile([NCORES * P, qsz[q]], f32, tag=f"h_table{q}",
                            name=f"h_table{q}", addr_space="Shared", bufs=2)
            nc.gpsimd.collective_compute(
                "AllGather", mybir.AluOpType.bypass, replica_groups=rg,
                ins=[h_shard[q][:, 0:qsz[q]].opt()],
                outs=[tab[:].opt()])
            h_table[q] = tab

        def shard_cols(blk, width):
            """node block (width cols from blk*width) -> (quarter, col off)"""
            lo = blk * width
            q = min(lo // cfg.qs, 3)
            return q, lo - q * cfg.qs

        # ---------------- init: h0 = relu(x@W_in + b_in) ----------------
        for nt in range(NB):
            x_tile = xiop.tile([P, in_dim], f32, tag="x", name="x_tile")
            nc.sync.dma_start(out=x_tile[:], in_=x_in[nt * P:(nt + 1) * P, :])
            xts = []
            for k in range(in_dim // P):
                xt_ps = ptr.tile([P, P], f32, tag="ptr", name="xt_ps")
                nc.tensor.transpose(xt_ps[:], x_tile[:, k * P:(k + 1) * P], id1[:])
                xt_sb = xtp.tile([P, P], f32, tag="xt", name="xt_sb")
                nc.vector.tensor_copy(out=xt_sb[:], in_=xt_ps[:])
                xts.append(xt_sb)
            ph0 = pmisc.tile([P, YB], f32, tag="pmisc", name="pm")
            nk = in_dim // P
            for k in range(nk):
                nc.tensor.matmul(ph0[:, :P], lhsT=w_in_sb[:, k * hid:(k + 1) * hid],
                                 rhs=xts[k][:], start=(k == 0), stop=(k == nk - 1))
            wb = wbp.tile([P, YB], f32, tag="wb", name="wb")
            nc.scalar.activation(wb[:, :P], ph0[:, :P], Relu, bias=b_in_sb[:])
            q, co = shard_cols(nt, P)
            take = min(P, qsz[q] - co)
            nc.sync.dma_start(out=h_shard[q][:, co:co + take], in_=wb[:, :take])
            h0t = h0sp.tile([P, P], f32, tag="h0t", name="h0t")
            nc.scalar.activation(h0t[:], ph0[:, :P], Relu, bias=b_in_s[:],
                                 scale=ALPHA)
            nc.sync.dma_start(out=h0s_dram[:, nt * P:(nt + 1) * P], in_=h0t[:])
            if q < 3 and nt == (q + 1) * (cfg.qs // P) - 1:
                emit_ag(q)
            elif nt == NB - 1:
                emit_ag(3)

        # ---------------- layers ----------------
        for layer in range(cfg.layers):
            last = layer == cfg.layers - 1
            tables = list(h_table)

            def finish_block(b):
                """512-wide output block b of s_acc is complete"""
                w = min(YB, cfg.n_pad - b * YB)
                cols = slice(b * YB, b * YB + w)
                ps = pmisc.tile([P, YB], f32, tag="pmisc", name="pm")
                nc.tensor.matmul(ps[:, :w],
                                 lhsT=w_eff_sb[:, layer * hid:(layer + 1) * hid],
                                 rhs=s_acc[:, cols], start=True, stop=True)
                if not last:
                    wb = wbp.tile([P, YB], f32, tag="wb", name="wb")
                    nc.scalar.activation(wb[:, :w], ps[:, :w], Relu)
                    done = 0
                    while done < w:       # may straddle quarter boundary
                        q = min((b * YB + done) // cfg.qs, 3)
                        co = b * YB + done - q * cfg.qs
                        take = min(w - done, qsz[q] - co)
                        if take <= 0:     # pad columns past the real nodes
                            break
                        nc.sync.dma_start(out=h_shard[q][:, co:co + take],
                                          in_=wb[:, done:done + take])
                        done += take
                    # fire AGs when a quarter's columns are all written
                    hi = b * YB + w
                    for q in range(3):
                        if b * YB < (q + 1) * cfg.qs <= hi:
                            emit_ag(q)
                    if hi == cfg.n_pad:
                        emit_ag(3)
                else:
                    h8 = wbp.tile([P, YB], f32, tag="wb", name="wb")
                    nc.scalar.activation(h8[:, :w], ps[:, :w], Relu)
                    psy = pmisc.tile([P, YB], f32, tag="pmisc", name="pm")
                    nc.tensor.matmul(psy[0:1, :w], lhsT=w_out_sb[:, 0:1],
                                     rhs=h8[:, :w], start=True, stop=True)
                    yt = ytp.tile([1, YB], f32, tag="yt", name="yt")
                    nc.vector.tensor_tensor(
                        out=yt[0:1, :w], in0=psy[0:1, :w],
                        in1=b_out_sb[0:1, 0:1].to_broadcast([1, w]),
                        op=mybir.AluOpType.add)
                    nc.sync.dma_start(out=y_out[0:1, b * YB:b * YB + w],
                                      in_=yt[0:1, :w])

            for c in range(NW):
                q = c // 2
                wsz = cfg.wsize(c)
                # load window: 4 rank blocks of the AG'd quarter table
                win = winp.tile([P, 4 * cfg.qs], f32, tag="win", name="win")
                for j in range(4):
                    rr = (c % 2) * 4 + j
                    nc.sync.dma_start(
                        out=win[:, j * qsz[q]:(j + 1) * qsz[q]],
                        in_=tables[q][rr * P:(rr + 1) * P, :])
                win3 = win[:, :wsz].rearrange("p (n d) -> p n d", d=1)

                Lc = int(L[c])
                nsg = (Lc + NI - 1) // NI
                mt_tiles = {}          # 128-slot group -> sbuf [slot, feat]
                s_tiles = [None] * ((Lc // P + SB - 1) // SB)

                def ensure_group(g, c=c, win3=win3, Lc=Lc, mt_tiles=mt_tiles,
                                 s_tiles=s_tiles):
                    """gather+transpose 128-slot group g; returns sbuf tiles"""
                    if g in mt_tiles:
                        return
                    sg = (g * P) // NI
                    slots = min(NI, Lc - sg * NI)
                    gt = gathp.tile([P, NI], f32, tag="gt", name="gt")
                    it = idxp.tile([P, NI // 16], i16, tag="it", name="it")
                    nc.sync.dma_start(
                        out=it[:, :slots // 16],
                        in_=idx_in[c][:, sg * (NI // 16):sg * (NI // 16) + slots // 16])
                    nc.gpsimd.ap_gather(
                        gt[:, :slots].rearrange("p (n d) -> p n d", d=1), win3,
                        it[:, :slots // 16], P, wsz, 1, slots)
                    sbi = (sg * NI) // (SB * P)
                    for gg in range(sg * (NI // P), sg * (NI // P) + slots // P):
                        ps = ptr.tile([P, P], f32, tag="ptr", name="tr_ps")
                        off = gg * P - sg * NI
                        nc.tensor.transpose(ps[:], gt[:, off:off + P], id1[:])
                        mt = mtp.tile([P, P], f32, tag="mt", name="mt")
                        nc.vector.tensor_copy(out=mt[:], in_=ps[:])
                        mt_tiles[gg] = mt
                    for sb in range(sbi, min(sbi + NI // (SB * P) + 1,
                                             len(s_tiles))):
                        if s_tiles[sb] is None:
                            st = sblkp.tile([P, SB * CW], f32, tag="st", name="st")
                            lo = sb * SB * CW
                            ncols = min(SB * CW, (Lc // P) * CW - lo)
                            nc.sync.dma_start(out=st[:, :ncols],
                                              in_=s_in[c][:, lo:lo + ncols])
                            s_tiles[sb] = st

                cur = 0
                for b in range(cfg.NC2):
                    ps_b = pagg.tile([P, CW], f32, tag="ps_b", name="ps_b")
                    n_slots = int(n_cb[c][b])
                    first = True
                    left = n_slots
                    while left > 0:
                        g, p0 = cur // P, cur % P
                        ln = 0
                        for sz in (128, 64, 32):
                            if p0 % sz == 0 and left >= sz and p0 + sz <= P:
                                ln = sz
                                break
                        assert ln, (p0, left)
                        ensure_group(g)
                        st = s_tiles[g // SB]
                        so = (g % SB) * CW
                        nc.tensor.matmul(
                            ps_b[:],
                            lhsT=mt_tiles[g][p0:p0 + ln, :],
                            rhs=st[p0:p0 + ln, so:so + CW],
                            start=first, stop=(ln == left),
                            tile_position=(p0, 0))
                        first = False
                        cur += ln
                        left -= ln
                    if c == 0:
                        h0t = h0sp.tile([P, CW], f32, tag="h0t", name="h0t")
                        nc.sync.dma_start(out=h0t[:],
                                          in_=h0s_dram[:, b * CW:(b + 1) * CW])
                        nc.vector.tensor_add(out=s_acc[:, b * CW:(b + 1) * CW],
                                             in0=h0t[:], in1=ps_b[:])
                    else:
                        nc.vector.tensor_add(out=s_acc[:, b * CW:(b + 1) * CW],
                                             in0=s_acc[:, b * CW:(b + 1) * CW],
                                             in1=ps_b[:])
                    if c == NW - 1 and (((b + 1) * CW) % YB == 0 or
                                        b == cfg.NC2 - 1):
                        finish_block((b * CW) // YB)

    nc.compile()
    return nc


# ---------------- top level ----------------

def _assemble_y(results, cfg):
    parts = []
    for r in range(NCORES):
        y = np.asarray(results[r]["y"], np.float32).reshape(-1)
        parts.append(y[:cfg.n_per])
    return np.concatenate(parts)


def _run_pjrt(nc, in_maps, n_cores, time_iters=0):
    """Execute the bass program on the NeuronCores via PJRT (the axon
    redirect path of run_bass_kernel_spmd), with inputs pre-staged on
    device.  Mirrors concourse.bass2jax.run_bass_via_pjrt (multi-core).

    The axon dispatch floor is ~80ms/call, so single-call wall time says
    nothing about device time; with time_iters > 0 the marginal cost per
    execute between pipelined batches of M_lo and M_hi back-to-back
    calls is reported: device exec time plus ~1ms per-call dispatch (an
    honest upper bound on HW time).
    """
    import time
    import jax
    from jax.sharding import Mesh, NamedSharding, PartitionSpec
    from jax.experimental.shard_map import shard_map
    from concourse import bass2jax, mybir

    bass2jax.install_neuronx_cc_hook()

    partition_name = nc.partition_id_tensor.name if nc.partition_id_tensor else None
    in_names, out_names, out_avals, zero_outs = [], [], [], []
    for alloc in nc.m.functions[0].allocations:
        if not isinstance(alloc, mybir.MemoryLocationSet):
            continue
        name = alloc.memorylocations[0].name
        if alloc.kind == "ExternalInput":
            if name != partition_name:
                in_names.append(name)
        elif alloc.kind == "ExternalOutput":
            out_names.append(name)
            shape = tuple(alloc.tensor_shape)
            dtype = mybir.dt.np(alloc.dtype)
            out_avals.append(jax.core.ShapedArray(shape, dtype))
            zero_outs.append(np.zeros(shape, dtype))
    n_params = len(in_names)
    n_outs = len(out_avals)
    in_names.extend(out_names)
    if partition_name is not None:
        in_names.append(partition_name)
    donate = tuple(range(n_params, n_params + n_outs))

    def _body(*args):
        operands = list(args)
        if partition_name is not None:
            operands.append(bass2jax.partition_id_tensor())
        outs = bass2jax._bass_exec_p.bind(
            *operands,
            out_avals=tuple(out_avals),
            in_names=tuple(in_names),
            out_names=tuple(out_names),
            lowering_input_output_aliases=(),
            sim_require_finite=True,
            sim_require_nnan=True,
            nc=nc,
        )
        return tuple(outs)

    devices = jax.devices()[:n_cores]
    assert len(devices) == n_cores
    mesh = Mesh(np.asarray(devices), ("core",))
    in_specs = (PartitionSpec("core"),) * (n_params + n_outs)
    out_specs = (PartitionSpec("core"),) * len(out_names)
    sharded = jax.jit(
        shard_map(_body, mesh=mesh, in_specs=in_specs, out_specs=out_specs,
                  check_rep=False),
        donate_argnums=donate, keep_unused=True)

    shard = NamedSharding(mesh, PartitionSpec("core"))
    concat_in = [
        jax.device_put(
            np.concatenate([np.asarray(in_maps[c][name]) for c in range(n_cores)],
                           axis=0), shard)
        for name in in_names[:n_params]
    ]
    jax.block_until_ready(concat_in)

    def zeros():
        return [
            jax.device_put(np.zeros((n_cores * z.shape[0], *z.shape[1:]), z.dtype),
                           shard)
            for z in zero_outs
        ]

    out_arrs = jax.block_until_ready(sharded(*concat_in, *zeros()))
    exec_ns = None
    if time_iters > 0:
        m_lo, m_hi = 4, 4 + max(4, time_iters)

        def run_m(m):
            zs = [zeros() for _ in range(m)]
            jax.block_until_ready(zs)
            t0 = time.perf_counter()
            rs = [sharded(*concat_in, *z) for z in zs]
            jax.block_until_ready(rs)
            return time.perf_counter() - t0

        run_m(2)  # warm
        lo = min(run_m(m_lo) for _ in range(2))
        hi = min(run_m(m_hi) for _ in range(2))
        exec_ns = int(max(hi - lo, 0) / (m_hi - m_lo) * 1e9)
    results = [
        {name: np.asarray(out_arrs[i]).reshape(n_cores, *out_avals[i].shape)[c]
         for i, name in enumerate(out_names)}
        for c in range(n_cores)
    ]
    return results, exec_ns


def kernel(x, edge_index, W_in, b_in, convs_W, W_out, b_out):
    global LAST_EXEC_NS
    cfg = _cfg_full()
    in_maps, meta = preprocess(x, edge_index, W_in, b_in, convs_W, W_out, b_out,
                               cfg)
    nc = build(cfg, meta)
    iters = int(os.environ.get("KERNEL_TIME_ITERS", "0"))
    results, exec_ns = _run_pjrt(nc, in_maps, NCORES, time_iters=iters)
    LAST_EXEC_NS = exec_ns
    return _assemble_y(results, cfg)


# revision 33
# speedup vs baseline: 1.0918x; 1.0593x over previous
"""GCNII node regressor on 8 trn2 NeuronCores (Bass/Tile kernel).

Strategy (per sharding_hint): nodes are row-sharded across the 8 cores
(12500 each); edges are partitioned by dst core so the segment-sum is
local; the small weights are replicated.  h lives in [feat, node]
layout.  Per layer each core all-gathers h in 4 source-quarter
sub-AllGathers (pipelined with compute); source windows (half-quarters,
12800 nodes) are DMA'd into SBUF and edge-source columns are gathered
in-SBUF with the stock ap_gather GPSIMD instruction (the runtime image
lacks the custom dma_gather ucode; ap_gather measures ~26ns/row, which
is the kernel's bottleneck).  Gathered message columns are transposed
128-at-a-time on the TensorEngine and aggregated with host-prebuilt
one-hot*weight "S" blocks as dense [K<=128 x 256 dst] matmuls into
PSUM: s = 0.9*Ahat@h + 0.1*h0 accumulates into a resident SBUF tile
[128 feat x nodes]; the layer update h+ = relu((1-b)s + b*s@W) is fused
into one matmul with W_eff = (1-b)I + b*W precomputed on host.

Everything irregular (degrees, edge normalization, sorting edges into
uniform per-(window, dst-block) cells, gather index / S-block streams)
is precomputed on the host in numpy; the device program is
straight-line fp32 and identical on all 8 cores (SPMD).
"""

import math
import os

import numpy as np

# ---------------- problem constants (full size, hardcoded) ----------------
N = 100000
E = 1600000
IN_DIM = 256
HID = 128
LAYERS = 8
ALPHA = 0.1
THETA = 0.5
NCORES = 8

P = 128          # partitions
NW = 4           # source windows per layer (quarters)
NI = 2048        # gather slots per ap_gather call
CW = 256         # aggregation cell dst width
YB = 512         # output block width

LAST_EXEC_NS = None


class Cfg:
    def __init__(self, n, e, in_dim, hid, layers):
        assert n % NCORES == 0
        self.n, self.e, self.in_dim, self.hid, self.layers = n, e, in_dim, hid, layers
        self.n_per = n // NCORES
        self.n_pad = ((self.n_per + P - 1) // P) * P
        # source quarters: q0..q2 of size qs (multiple of 128), q3 remainder
        qs = ((self.n_per + 3) // 4 + P - 1) // P * P
        self.qs = qs
        q3 = self.n_per - 3 * qs
        assert 0 < q3 <= qs, (self.n_per, qs, q3)
        self.qsizes = [qs, qs, qs, q3]          # real rows per rank-quarter
        for sz in self.qsizes:
            assert 8 * sz <= 32767, "window slot must fit int16"
            assert 8 * sz * 1 <= 2 ** 15, "ap_gather num_elems limit"
        self.NB = self.n_pad // P               # dst 128-blocks per core
        self.NC2 = self.n_pad // CW             # dst cell blocks per core
        assert 3 * qs % P == 0
        self.betas = [float(np.log(THETA / (i + 1) + 1.0)) for i in range(layers)]

    def wsize(self, w):
        """window w (= quarter) node count"""
        return 8 * self.qsizes[w]


def _cfg_full():
    return Cfg(N, E, IN_DIM, HID, LAYERS)


# ---------------- host preprocessing ----------------

def preprocess(x, edge_index, W_in, b_in, convs_W, W_out, b_out, cfg):
    """Build per-core input maps + shared structure metadata."""
    n, n_per, qs = cfg.n, cfg.n_per, cfg.qs
    qsz = np.asarray(cfg.qsizes, np.int64)
    row = np.asarray(edge_index[0], np.int64)
    col = np.asarray(edge_index[1], np.int64)

    deg = np.bincount(col, minlength=n).astype(np.float32) + 1.0
    dinv = (1.0 / np.sqrt(deg)).astype(np.float32)
    wt = ((1.0 - ALPHA) * dinv[row] * dinv[col]).astype(np.float32)

    # append self loops as explicit edges
    allv = np.arange(n, dtype=np.int64)
    row_a = np.concatenate([row, allv])
    col_a = np.concatenate([col, allv])
    wt_a = np.concatenate([wt, ((1.0 - ALPHA) * dinv * dinv).astype(np.float32)])

    # source window + slot within window
    r_s = row_a // n_per
    i_s = row_a % n_per
    q_s = np.minimum(i_s // qs, 3)
    w_s = q_s
    slot = (r_s * qsz[q_s] + (i_s - q_s * qs)).astype(np.int64)

    r_d = col_a // n_per
    dloc = col_a % n_per
    b_idx = dloc // CW                           # dst cell block
    dcol = (dloc % CW).astype(np.int64)

    NB = cfg.NC2
    counts = np.zeros((NCORES, NW, NB), np.int64)
    np.add.at(counts, (r_d, w_s, b_idx), 1)
    n_cb = counts.max(axis=0)                    # [NW, NB]
    n_cb = np.maximum(32 * ((n_cb + 31) // 32), 32)
    L = n_cb.sum(axis=1)
    n_cb[:, NB - 1] += (-L) % P                  # chunk streams multiple of 128
    L = n_cb.sum(axis=1)                         # [NW]

    offs = np.zeros((NW, NB + 1), np.int64)
    offs[:, 1:] = np.cumsum(n_cb, axis=1)

    key = (r_d * NW + w_s) * NB + b_idx
    order = np.argsort(key, kind="stable")
    sk = key[order]
    grp_first = np.r_[0, np.flatnonzero(np.diff(sk)) + 1]
    grp_id = np.zeros(len(sk), np.int64)
    grp_id[grp_first[1:]] = 1
    grp_id = np.cumsum(grp_id)
    rank_in_cell = np.arange(len(sk)) - grp_first[grp_id]
    pos = offs[w_s[order], b_idx[order]] + rank_in_cell

    in_maps = []
    for r in range(NCORES):
        m = {}
        xs = np.zeros((cfg.n_pad, cfg.in_dim), np.float32)
        xs[:n_per] = np.asarray(x[r * n_per:(r + 1) * n_per], np.float32)
        m["x"] = xs
        sel_r = r_d[order] == r
        for c in range(NW):
            sel = sel_r & (w_s[order] == c)
            p = pos[sel]
            idx_stream = np.zeros(L[c], np.int16)
            idx_stream[p] = slot[order][sel].astype(np.int16)
            sarr = np.zeros((P, (L[c] // P) * CW), np.float32)
            sarr[p % P, (p // P) * CW + dcol[order][sel]] = wt_a[order][sel]
            idxd = np.tile(idx_stream.reshape(-1, 16).T, (8, 1))
            m[f"idx{c}"] = np.ascontiguousarray(idxd)
            m[f"sblk{c}"] = sarr
        m["w_in"] = np.asarray(W_in, np.float32)
        m["b_in"] = np.asarray(b_in, np.float32).reshape(cfg.hid, 1)
        weff = np.concatenate(
            [((1.0 - cfg.betas[i]) * np.eye(cfg.hid, dtype=np.float32)
              + cfg.betas[i] * np.asarray(convs_W[i], np.float32))
             for i in range(cfg.layers)], axis=1)
        m["w_eff"] = weff
        m["w_out"] = np.asarray(W_out, np.float32).reshape(cfg.hid, 1)
        m["b_out"] = np.asarray(b_out, np.float32).reshape(1, 1)
        in_maps.append(m)

    return in_maps, {"n_cb": n_cb, "L": L}


# ---------------- device program ----------------

def build(cfg, meta, debug=False):
    import concourse.bass as bass
    import concourse.mybir as mybir
    from concourse import bacc
    from concourse.masks import make_identity
    from concourse.tile import TileContext
    from contextlib import ExitStack

    f32 = mybir.dt.float32
    i16 = mybir.dt.int16
    Relu = mybir.ActivationFunctionType.Relu
    n_cb, L = meta["n_cb"], meta["L"]
    hid, in_dim = cfg.hid, cfg.in_dim
    NB = cfg.NB
    qsz = cfg.qsizes
    SB = 8                   # S groups per stream tile

    nc = bacc.Bacc("TRN2", target_bir_lowering=False, debug=debug)

    x_in = nc.dram_tensor("x", [cfg.n_pad, in_dim], f32, kind="ExternalInput")
    idx_in, s_in = [], []
    for c in range(NW):
        idx_in.append(nc.dram_tensor(f"idx{c}", [P, int(L[c]) // 16], i16,
                                     kind="ExternalInput"))
        s_in.append(nc.dram_tensor(f"sblk{c}", [P, (int(L[c]) // P) * CW], f32,
                                   kind="ExternalInput"))
    w_in_t = nc.dram_tensor("w_in", [in_dim, hid], f32, kind="ExternalInput")
    b_in_t = nc.dram_tensor("b_in", [hid, 1], f32, kind="ExternalInput")
    w_eff_t = nc.dram_tensor("w_eff", [hid, cfg.layers * hid], f32,
                             kind="ExternalInput")
    w_out_t = nc.dram_tensor("w_out", [hid, 1], f32, kind="ExternalInput")
    b_out_t = nc.dram_tensor("b_out", [1, 1], f32, kind="ExternalInput")
    y_out = nc.dram_tensor("y", [1, cfg.n_pad], f32, kind="ExternalOutput")

    rg = [list(range(NCORES))]

    with TileContext(nc) as tc, ExitStack() as ctx:
        const = ctx.enter_context(tc.tile_pool(name="const", bufs=1))
        sfp = ctx.enter_context(tc.tile_pool(name="sfp", bufs=2))
        winp = ctx.enter_context(tc.tile_pool(name="winp", bufs=1))
        gathp = ctx.enter_context(tc.tile_pool(name="gath", bufs=3))
        mtp = ctx.enter_context(tc.tile_pool(name="mtp", bufs=20))
        sblkp = ctx.enter_context(tc.tile_pool(name="sblk", bufs=3))
        idxp = ctx.enter_context(tc.tile_pool(name="idxt", bufs=3))
        xiop = ctx.enter_context(tc.tile_pool(name="xio", bufs=2))
        xtp = ctx.enter_context(tc.tile_pool(name="xt", bufs=3))
        wbp = ctx.enter_context(tc.tile_pool(name="wb", bufs=3))
        h0sp = ctx.enter_context(tc.tile_pool(name="h0sp", bufs=3))
        ytp = ctx.enter_context(tc.tile_pool(name="yt", bufs=2))
        pagg = ctx.enter_context(tc.tile_pool(name="pagg", bufs=3, space="PSUM"))
        ptr = ctx.enter_context(tc.tile_pool(name="ptr", bufs=3, space="PSUM"))
        pmisc = ctx.enter_context(tc.tile_pool(name="pmisc", bufs=2, space="PSUM"))
        dram = ctx.enter_context(tc.tile_pool(name="dram", bufs=1, space="DRAM"))

        # per-quarter h shard in [feat, node] layout + AG'd tables
        h_shard = []
        h_table = [None] * 4
        for q in range(4):
            h_shard.append(dram.tile([P, qsz[q]], f32, tag=f"h_shard{q}",
                                     name=f"h_shard{q}"))
        h0s_dram = dram.tile([P, cfg.n_pad], f32, tag="h0s", name="h0s_dram")

        id1 = const.tile([P, P], f32, tag="id1", name="id1")
        make_identity(nc, id1[:])
        w_in_sb = const.tile([P, (in_dim // P) * hid], f32, tag="w_in",
                             name="w_in_sb")
        for k in range(in_dim // P):
            nc.sync.dma_start(out=w_in_sb[:, k * hid:(k + 1) * hid],
                              in_=w_in_t[k * P:(k + 1) * P, :])
        b_in_sb = const.tile([P, 1], f32, tag="b_in", name="b_in_sb")
        nc.sync.dma_start(out=b_in_sb[:], in_=b_in_t[:])
        b_in_s = const.tile([P, 1], f32, tag="b_in_s", name="b_in_s")
        nc.vector.tensor_scalar_mul(b_in_s[:], b_in_sb[:], ALPHA)
        w_eff_sb = const.tile([P, cfg.layers * hid], f32, tag="w_eff",
                              name="w_eff_sb")
        nc.sync.dma_start(out=w_eff_sb[:], in_=w_eff_t[:])
        w_out_sb = const.tile([P, 1], f32, tag="w_out", name="w_out_sb")
        nc.sync.dma_start(out=w_out_sb[:], in_=w_out_t[:])
        b_out_sb = const.tile([1, 1], f32, tag="b_out", name="b_out_sb")
        nc.sync.dma_start(out=b_out_sb[:], in_=b_out_t[:])

        s_acc = dram.tile([P, cfg.n_pad], f32, tag="s_acc", name="s_acc")

        reg_cache = {}

        def nreg(v):
            if v not in reg_cache:
                reg_cache[v] = nc.gpsimd.to_reg(v)
            return reg_cache[v]

        def emit_ag(q):
            tab = dram.tile([NCORES * P, qsz[q]], f32, tag=f"h_table{q}",
                            name=f"h_table{q}", addr_space="Shared", bufs=2)
            nc.gpsimd.collective_compute(
                "AllGather", mybir.AluOpType.bypass, replica_groups=rg,
                ins=[h_shard[q][:, 0:qsz[q]].opt()],
                outs=[tab[:].opt()])
            h_table[q] = tab

        def shard_cols(blk, width):
            """node block (width cols from blk*width) -> (quarter, col off)"""
            lo = blk * width
            q = min(lo // cfg.qs, 3)
            return q, lo - q * cfg.qs

        # ---------------- init: h0 = relu(x@W_in + b_in) ----------------
        for nt in range(NB):
            x_tile = xiop.tile([P, in_dim], f32, tag="x", name="x_tile")
            nc.sync.dma_start(out=x_tile[:], in_=x_in[nt * P:(nt + 1) * P, :])
            xts = []
            for k in range(in_dim // P):
                xt_ps = ptr.tile([P, P], f32, tag="ptr", name="xt_ps")
                nc.tensor.transpose(xt_ps[:], x_tile[:, k * P:(k + 1) * P], id1[:])
                xt_sb = xtp.tile([P, P], f32, tag="xt", name="xt_sb")
                nc.vector.tensor_copy(out=xt_sb[:], in_=xt_ps[:])
                xts.append(xt_sb)
            ph0 = pmisc.tile([P, YB], f32, tag="pmisc", name="pm")
            nk = in_dim // P
            for k in range(nk):
                nc.tensor.matmul(ph0[:, :P], lhsT=w_in_sb[:, k * hid:(k + 1) * hid],
                                 rhs=xts[k][:], start=(k == 0), stop=(k == nk - 1))
            wb = wbp.tile([P, YB], f32, tag="wb", name="wb")
            nc.scalar.activation(wb[:, :P], ph0[:, :P], Relu, bias=b_in_sb[:])
            q, co = shard_cols(nt, P)
            take = min(P, qsz[q] - co)
            nc.sync.dma_start(out=h_shard[q][:, co:co + take], in_=wb[:, :take])
            h0t = h0sp.tile([P, P], f32, tag="h0t", name="h0t")
            nc.scalar.activation(h0t[:], ph0[:, :P], Relu, bias=b_in_s[:],
                                 scale=ALPHA)
            nc.sync.dma_start(out=h0s_dram[:, nt * P:(nt + 1) * P], in_=h0t[:])
            if q < 3 and nt == (q + 1) * (cfg.qs // P) - 1:
                emit_ag(q)
            elif nt == NB - 1:
                emit_ag(3)

        # ---------------- layers ----------------
        for layer in range(cfg.layers):
            last = layer == cfg.layers - 1
            tables = list(h_table)

            def finish_block(b):
                """512-wide output block b of s_acc is complete"""
                w = min(YB, cfg.n_pad - b * YB)
                cols = slice(b * YB, b * YB + w)
                sf = sfp.tile([P, YB], f32, tag="sf", name="sf")
                nc.sync.dma_start(out=sf[:, :w], in_=s_acc[:, cols])
                ps = pmisc.tile([P, YB], f32, tag="pmisc", name="pm")
                nc.tensor.matmul(ps[:, :w],
                                 lhsT=w_eff_sb[:, layer * hid:(layer + 1) * hid],
                                 rhs=sf[:, :w], start=True, stop=True)
                if not last:
                    wb = wbp.tile([P, YB], f32, tag="wb", name="wb")
                    nc.scalar.activation(wb[:, :w], ps[:, :w], Relu)
                    done = 0
                    while done < w:       # may straddle quarter boundary
                        q = min((b * YB + done) // cfg.qs, 3)
                        co = b * YB + done - q * cfg.qs
                        take = min(w - done, qsz[q] - co)
                        if take <= 0:     # pad columns past the real nodes
                            break
                        nc.sync.dma_start(out=h_shard[q][:, co:co + take],
                                          in_=wb[:, done:done + take])
                        done += take
                    # fire AGs when a quarter's columns are all written
                    hi = b * YB + w
                    for q in range(3):
                        if b * YB < (q + 1) * cfg.qs <= hi:
                            emit_ag(q)
                    if hi == cfg.n_pad:
                        emit_ag(3)
                else:
                    h8 = wbp.tile([P, YB], f32, tag="wb", name="wb")
                    nc.scalar.activation(h8[:, :w], ps[:, :w], Relu)
                    psy = pmisc.tile([P, YB], f32, tag="pmisc", name="pm")
                    nc.tensor.matmul(psy[0:1, :w], lhsT=w_out_sb[:, 0:1],
                                     rhs=h8[:, :w], start=True, stop=True)
                    yt = ytp.tile([1, YB], f32, tag="yt", name="yt")
                    nc.vector.tensor_tensor(
                        out=yt[0:1, :w], in0=psy[0:1, :w],
                        in1=b_out_sb[0:1, 0:1].to_broadcast([1, w]),
                        op=mybir.AluOpType.add)
                    nc.sync.dma_start(out=y_out[0:1, b * YB:b * YB + w],
                                      in_=yt[0:1, :w])

            for c in range(NW):
                q = c
                wsz = cfg.wsize(c)
                # load window: all 8 rank blocks of the AG'd quarter table
                win = winp.tile([P, 8 * cfg.qs], f32, tag="win", name="win")
                for rr in range(8):
                    nc.sync.dma_start(
                        out=win[:, rr * qsz[q]:(rr + 1) * qsz[q]],
                        in_=tables[q][rr * P:(rr + 1) * P, :])
                win3 = win[:, :wsz].rearrange("p (n d) -> p n d", d=1)

                Lc = int(L[c])
                nsg = (Lc + NI - 1) // NI
                mt_tiles = {}          # 128-slot group -> sbuf [slot, feat]
                s_tiles = [None] * ((Lc // P + SB - 1) // SB)

                def ensure_group(g, c=c, win3=win3, Lc=Lc, mt_tiles=mt_tiles,
                                 s_tiles=s_tiles):
                    """gather+transpose 128-slot group g; returns sbuf tiles"""
                    if g in mt_tiles:
                        return
                    sg = (g * P) // NI
                    slots = min(NI, Lc - sg * NI)
                    gt = gathp.tile([P, NI], f32, tag="gt", name="gt")
                    it = idxp.tile([P, NI // 16], i16, tag="it", name="it")
                    nc.sync.dma_start(
                        out=it[:, :slots // 16],
                        in_=idx_in[c][:, sg * (NI // 16):sg * (NI // 16) + slots // 16])
                    nc.gpsimd.ap_gather(
                        gt[:, :slots].rearrange("p (n d) -> p n d", d=1), win3,
                        it[:, :slots // 16], P, wsz, 1, slots)
                    sbi = (sg * NI) // (SB * P)
                    for gg in range(sg * (NI // P), sg * (NI // P) + slots // P):
                        ps = ptr.tile([P, P], f32, tag="ptr", name="tr_ps")
                        off = gg * P - sg * NI
                        nc.tensor.transpose(ps[:], gt[:, off:off + P], id1[:])
                        mt = mtp.tile([P, P], f32, tag="mt", name="mt")
                        nc.vector.tensor_copy(out=mt[:], in_=ps[:])
                        mt_tiles[gg] = mt
                    for sb in range(sbi, min(sbi + NI // (SB * P) + 1,
                                             len(s_tiles))):
                        if s_tiles[sb] is None:
                            st = sblkp.tile([P, SB * CW], f32, tag="st", name="st")
                            lo = sb * SB * CW
                            ncols = min(SB * CW, (Lc // P) * CW - lo)
                            nc.sync.dma_start(out=st[:, :ncols],
                                              in_=s_in[c][:, lo:lo + ncols])
                            s_tiles[sb] = st

                cur = 0
                for b in range(cfg.NC2):
                    ps_b = pagg.tile([P, CW], f32, tag="ps_b", name="ps_b")
                    n_slots = int(n_cb[c][b])
                    first = True
                    left = n_slots
                    while left > 0:
                        g, p0 = cur // P, cur % P
                        ln = 0
                        for sz in (128, 64, 32):
                            if p0 % sz == 0 and left >= sz and p0 + sz <= P:
                                ln = sz
                                break
                        assert ln, (p0, left)
                        ensure_group(g)
                        st = s_tiles[g // SB]
                        so = (g % SB) * CW
                        nc.tensor.matmul(
                            ps_b[:],
                            lhsT=mt_tiles[g][p0:p0 + ln, :],
                            rhs=st[p0:p0 + ln, so:so + CW],
                            start=first, stop=(ln == left),
                            tile_position=(p0, 0))
                        first = False
                        cur += ln
                        left -= ln
                    sa = h0sp.tile([P, CW], f32, tag="h0t", name="sa")
                    src_t = h0s_dram if c == 0 else s_acc
                    nc.sync.dma_start(out=sa[:],
                                      in_=src_t[:, b * CW:(b + 1) * CW])
                    nc.vector.tensor_add(out=sa[:], in0=sa[:], in1=ps_b[:])
                    nc.sync.dma_start(out=s_acc[:, b * CW:(b + 1) * CW],
                                      in_=sa[:])
                    if c == NW - 1 and (((b + 1) * CW) % YB == 0 or
                                        b == cfg.NC2 - 1):
                        finish_block((b * CW) // YB)

    nc.compile()
    return nc


# ---------------- top level ----------------

def _assemble_y(results, cfg):
    parts = []
    for r in range(NCORES):
        y = np.asarray(results[r]["y"], np.float32).reshape(-1)
        parts.append(y[:cfg.n_per])
    return np.concatenate(parts)


def _run_pjrt(nc, in_maps, n_cores, time_iters=0):
    """Execute the bass program on the NeuronCores via PJRT (the axon
    redirect path of run_bass_kernel_spmd), with inputs pre-staged on
    device.  Mirrors concourse.bass2jax.run_bass_via_pjrt (multi-core).

    The axon dispatch floor is ~80ms/call, so single-call wall time says
    nothing about device time; with time_iters > 0 the marginal cost per
    execute between pipelined batches of M_lo and M_hi back-to-back
    calls is reported: device exec time plus ~1ms per-call dispatch (an
    honest upper bound on HW time).
    """
    import time
    import jax
    from jax.sharding import Mesh, NamedSharding, PartitionSpec
    from jax.experimental.shard_map import shard_map
    from concourse import bass2jax, mybir

    bass2jax.install_neuronx_cc_hook()

    partition_name = nc.partition_id_tensor.name if nc.partition_id_tensor else None
    in_names, out_names, out_avals, zero_outs = [], [], [], []
    for alloc in nc.m.functions[0].allocations:
        if not isinstance(alloc, mybir.MemoryLocationSet):
            continue
        name = alloc.memorylocations[0].name
        if alloc.kind == "ExternalInput":
            if name != partition_name:
                in_names.append(name)
        elif alloc.kind == "ExternalOutput":
            out_names.append(name)
            shape = tuple(alloc.tensor_shape)
            dtype = mybir.dt.np(alloc.dtype)
            out_avals.append(jax.core.ShapedArray(shape, dtype))
            zero_outs.append(np.zeros(shape, dtype))
    n_params = len(in_names)
    n_outs = len(out_avals)
    in_names.extend(out_names)
    if partition_name is not None:
        in_names.append(partition_name)
    donate = tuple(range(n_params, n_params + n_outs))

    def _body(*args):
        operands = list(args)
        if partition_name is not None:
            operands.append(bass2jax.partition_id_tensor())
        outs = bass2jax._bass_exec_p.bind(
            *operands,
            out_avals=tuple(out_avals),
            in_names=tuple(in_names),
            out_names=tuple(out_names),
            lowering_input_output_aliases=(),
            sim_require_finite=True,
            sim_require_nnan=True,
            nc=nc,
        )
        return tuple(outs)

    devices = jax.devices()[:n_cores]
    assert len(devices) == n_cores
    mesh = Mesh(np.asarray(devices), ("core",))
    in_specs = (PartitionSpec("core"),) * (n_params + n_outs)
    out_specs = (PartitionSpec("core"),) * len(out_names)
    sharded = jax.jit(
        shard_map(_body, mesh=mesh, in_specs=in_specs, out_specs=out_specs,
                  check_rep=False),
        donate_argnums=donate, keep_unused=True)

    shard = NamedSharding(mesh, PartitionSpec("core"))
    concat_in = [
        jax.device_put(
            np.concatenate([np.asarray(in_maps[c][name]) for c in range(n_cores)],
                           axis=0), shard)
        for name in in_names[:n_params]
    ]
    jax.block_until_ready(concat_in)

    def zeros():
        return [
            jax.device_put(np.zeros((n_cores * z.shape[0], *z.shape[1:]), z.dtype),
                           shard)
            for z in zero_outs
        ]

    out_arrs = jax.block_until_ready(sharded(*concat_in, *zeros()))
    exec_ns = None
    if time_iters > 0:
        m_lo, m_hi = 4, 4 + max(4, time_iters)

        def run_m(m):
            zs = [zeros() for _ in range(m)]
            jax.block_until_ready(zs)
            t0 = time.perf_counter()
            rs = [sharded(*concat_in, *z) for z in zs]
            jax.block_until_ready(rs)
            return time.perf_counter() - t0

        run_m(2)  # warm
        lo = min(run_m(m_lo) for _ in range(2))
        hi = min(run_m(m_hi) for _ in range(2))
        exec_ns = int(max(hi - lo, 0) / (m_hi - m_lo) * 1e9)
    results = [
        {name: np.asarray(out_arrs[i]).reshape(n_cores, *out_avals[i].shape)[c]
         for i, name in enumerate(out_names)}
        for c in range(n_cores)
    ]
    return results, exec_ns


def kernel(x, edge_index, W_in, b_in, convs_W, W_out, b_out):
    global LAST_EXEC_NS
    cfg = _cfg_full()
    in_maps, meta = preprocess(x, edge_index, W_in, b_in, convs_W, W_out, b_out,
                               cfg)
    nc = build(cfg, meta)
    iters = int(os.environ.get("KERNEL_TIME_ITERS", "0"))
    results, exec_ns = _run_pjrt(nc, in_maps, NCORES, time_iters=iters)
    LAST_EXEC_NS = exec_ns
    return _assemble_y(results, cfg)
